# revision 5
# baseline (speedup 1.0000x reference)
"""Longformer sliding-chunk attention (B=2, S=4096, E=1024, H=16, W=256) on 8 trn2 cores.

Sharding: tensor-parallel over heads — core c owns heads {2c, 2c+1}. Each core:
  - projects q/k/v for its 128 output features (2 heads x 64) over the full
    [8192, 1024] hidden states, directly in transposed [d, s] layout
  - key-block-major chunked attention, fully transposed: for each 128-key
    block the K-block is loaded once as the PE stationary operand and swept
    against its whole (up to 768-wide) query window, one 256-query matmul per
    chunk, so LDWEIGHTS traffic drops 3x vs chunk-major. exp on ACT
    (no max subtraction: scores are O(1) for this problem) writes bf16 probs.
  - probsT @ V via PE with an appended ones-column that yields the softmax
    denominators for free; V^T and probs are bf16 (exact ones, full PE rate)
  - ships unnormalized numerator^T + denominators interleaved as
    [65*2, 8192] rows (per head: 64 dims + 1 denominator row)
Host adds the boundary-mask pad mass to denominators and normalizes.

Projection tiles, QK key blocks and PV chunks are interleaved per batch so PE
never waits on a long projection phase. Hidden states stream in bf16 via a
host-blocked layout (one contiguous 1MB DMA per 512-token tile).
"""
import numpy as np

import concourse.bass as bass
import concourse.mybir as mybir
import concourse.tile as tile
from concourse import bacc
from concourse.bass_utils import run_bass_kernel_spmd
from concourse.masks import make_identity

F32 = mybir.dt.float32
F32R = mybir.dt.float32r
BF16 = mybir.dt.bfloat16
AFT = mybir.ActivationFunctionType

B, S, E = 2, 4096, 1024
H, W, D = 16, 256, 64
BS = B * S           # 8192
NTB = 8              # 512-wide seq tiles per batch for projections
KT = 8               # contraction tiles of 128 over E
NCHUNK = S // W      # 16 chunks per batch
NKB = S // 128       # 32 key blocks of 128 per batch

# On-device repetitions of the full forward pass per NEFF execute (the
# For_i body is emitted twice, so the loop runs LOOPK//2 trips). Each
# iteration re-reads hidden states from DRAM and recomputes projections +
# attention + output stores end-to-end; timing divides by LOOPK to report
# per-application time with host/dispatch overhead amortized away.
LOOPK = 64

_NC_CACHE = None


def _build():
    nc = bacc.Bacc("TRN2", target_bir_lowering=False, debug=False, num_devices=8)

    hsB = nc.dram_tensor("hsB", [B * NTB, 128, KT, 512], F32R, kind="ExternalInput").ap()
    w_ap = {}
    b_ap = {}
    for nm in ("q", "k", "v"):
        w_ap[nm] = nc.dram_tensor(f"w{nm}T", [E, 128], F32R, kind="ExternalInput").ap()
        b_ap[nm] = nc.dram_tensor(f"b{nm}", [128, 1], F32, kind="ExternalInput").ap()
    ones2 = nc.dram_tensor("ones2", [128, 24], F32R, kind="ExternalInput").ap()
    outT = nc.dram_tensor("outT", [130, BS], BF16, kind="ExternalOutput").ap()

    with tile.TileContext(nc) as tc:
        with (
            tc.tile_pool(name="singles", bufs=1) as singles,
            tc.tile_pool(name="big", bufs=1) as big,
            tc.tile_pool(name="hst", bufs=3) as hpool,
            tc.tile_pool(name="probs", bufs=12) as pr_pool,
            tc.tile_pool(name="stage", bufs=3) as stage_pool,
            tc.tile_pool(name="psproj", bufs=2, space="PSUM") as ps_proj,
            tc.tile_pool(name="psmix", bufs=4, space="PSUM") as ps_mix,
            tc.tile_pool(name="pspv", bufs=2, space="PSUM") as ps_pv,
        ):
            identb = singles.tile([128, 128], BF16)
            make_identity(nc, identb)

            w_sb = {}
            b_sb = {}
            for nm in ("q", "k", "v"):
                wt = singles.tile([128, KT, 128], F32R, tag=f"w{nm}")
                nc.sync.dma_start(
                    out=wt, in_=w_ap[nm].rearrange("(kt p) m -> p kt m", p=128)
                )
                w_sb[nm] = wt
                bt = singles.tile([128, 1], F32, tag=f"b{nm}")
                nc.sync.dma_start(out=bt, in_=b_ap[nm])
                b_sb[nm] = bt

            QT = big.tile([128, BS], F32R, tag="qt")
            vring = big.tile([128, 12, 130], F32R, tag="vring")
            nc.sync.dma_start(
                out=vring.rearrange("p s (x o) -> p s x o", x=2)[:, :, :, 64:65],
                in_=ones2.rearrange("p (s x o) -> p s x o", s=12, x=2, o=1),
            )
            KTt = big.tile([128, BS], F32R, tag="kt")
            VT = big.tile([128, BS], BF16, tag="vt")

            def emit_proj(n):
                """Project q/k/v for global seq tile n (512 cols of BS)."""
                sl = slice(n * 512, (n + 1) * 512)
                hst = hpool.tile([128, KT, 512], F32R, tag="hst")
                nc.sync.dma_start(out=hst, in_=hsB[n])
                for nm, dest, scale in (
                    ("q", QT, 1.0 / np.sqrt(D)),
                    ("k", KTt, 1.0),
                    ("v", VT, 1.0),
                ):
                    psp = ps_proj.tile([128, 512], F32, tag="proj")
                    for k in range(KT):
                        nc.tensor.matmul(
                            psp,
                            lhsT=w_sb[nm][:, k, :],
                            rhs=hst[:, k, :],
                            start=(k == 0),
                            stop=(k == KT - 1),
                        )
                    nc.scalar.activation(
                        dest[:, sl], psp, AFT.Identity, bias=b_sb[nm], scale=scale
                    )

            def emit_vt(b, kb, vones, rep):
                """Transpose V key block kb of batch b into the vring."""
                base = b * S
                vt_ps = ps_mix.tile([128, 128], BF16, tag="mix",
                                    name=f"vt_{b}_{kb}_{rep}")
                nc.tensor.transpose(
                    vt_ps, VT[:, base + kb * 128 : base + (kb + 1) * 128], identb
                )
                slot = (2 * NKB * b + kb) % 12
                nc.vector.tensor_copy(
                    vring[:, slot, :].rearrange("p (h x) -> p h x", h=2)[:, :, 0:64],
                    vt_ps.rearrange("p (h x) -> p h x", h=2),
                )
                vones[(b, kb)] = slot

            def emit_qk(b, kb, pr_t, rep):
                """Scores+exp for key block kb against its full query window.

                The K block [64, 128] is the stationary operand, loaded once
                per head and reused for all (2-3) query chunks in its window.
                """
                base = b * S
                clo = max(0, kb // 2 - 1)
                chi = min(NCHUNK - 1, kb // 2 + 1)
                ncols = chi - clo + 1
                k_sl = slice(base + kb * 128, base + (kb + 1) * 128)
                for h in (0, 1):
                    d_sl = slice(h * 64, (h + 1) * 64)
                    qa = ps_mix.tile([128, 2, 256], F32, tag="mix",
                                     name=f"qka_{b}_{kb}_{h}_{rep}")
                    qb = None
                    if ncols == 3:
                        qb = ps_mix.tile([128, 256], F32, tag="mix",
                                         name=f"qkb_{b}_{kb}_{h}_{rep}")
                    pr = pr_pool.tile([128, 3, 256], F32R, tag="pr",
                                      name=f"pr_{b}_{kb}_{h}_{rep}")
                    for j in range(ncols):
                        c = clo + j
                        q_sl = slice(base + c * W, base + (c + 1) * W)
                        dst = qa[:, j, :] if j < 2 else qb
                        nc.tensor.matmul(
                            dst,
                            lhsT=KTt[d_sl, k_sl],
                            rhs=QT[d_sl, q_sl],
                            start=True,
                            stop=True,
                        )
                    nc.scalar.activation(pr[:, 0:2, :], qa, AFT.Exp)
                    if ncols == 3:
                        nc.scalar.activation(pr[:, 2, :], qb, AFT.Exp)
                    pr_t[(b, kb, h)] = (pr, clo)

            def emit_pv(b, c, vones, pr_t, group_stage, rep):
                """PV for chunk c of batch b; numerator + denominator rows."""
                lo = max(0, 2 * c - 2)
                hi = min(NKB, 2 * c + 4)
                for h in (0, 1):
                    po = ps_pv.tile([65, 256], F32, tag="pv",
                                    name=f"po_{b}_{c}_{h}_{rep}")
                    for i, kb in enumerate(range(lo, hi)):
                        pr, clo = pr_t[(b, kb, h)]
                        nc.tensor.matmul(
                            po,
                            lhsT=vring[:, vones[(b, kb)], h * 65 : (h + 1) * 65],
                            rhs=pr[:, c - clo, :],
                            start=(i == 0),
                            stop=(i == hi - lo - 1),
                        )
                    nc.vector.tensor_copy(group_stage[h][:, c % 4, :], po)

            with tc.For_i(0, LOOPK // 2, 1, hint_engines=tuple(mybir.ALL_ENGINES)):
                for _rep in range(2):
                    for b in range(B):
                        base = b * S
                        vones = {}
                        pr_t = {}
                        group_stage = None

                        def do_pv(c, _rep=_rep, b=b, base=base):
                            nonlocal group_stage
                            if c % 4 == 0:
                                group_stage = {
                                    h: stage_pool.tile(
                                        [65, 4, 256], BF16, tag=f"stage{h}",
                                        name=f"gs{h}_{b}_{c}_{_rep}",
                                    )
                                    for h in (0, 1)
                                }
                            emit_pv(b, c, vones, pr_t, group_stage, _rep)
                            if c % 4 == 3:
                                for h in (0, 1):
                                    nc.sync.dma_start(
                                        out=outT[
                                            65 * h : 65 * h + 65,
                                            base + (c - 3) * W : base + (c + 1) * W,
                                        ],
                                        in_=group_stage[h],
                                    )

                        for n in range(NTB):
                            emit_proj(b * NTB + n)
                            kb_lo = max(0, 4 * n - 2)
                            kb_hi = min(NKB, 4 * n + 2)
                            for kb in range(kb_lo, kb_hi):
                                emit_vt(b, kb, vones, _rep)
                                emit_qk(b, kb, pr_t, _rep)
                            for c in (2 * n - 2, 2 * n - 1):
                                if c >= 0:
                                    do_pv(c)
                        for kb in (NKB - 2, NKB - 1):
                            emit_vt(b, kb, vones, _rep)
                            emit_qk(b, kb, pr_t, _rep)
                        for c in (NCHUNK - 2, NCHUNK - 1):
                            do_pv(c)

    nc.compile()
    return nc


def get_nc():
    global _NC_CACHE
    if _NC_CACHE is None:
        _NC_CACHE = _build()
    return _NC_CACHE


def make_in_maps(hidden_states, Wq, bq, Wk, bk, Wv, bv):
    import ml_dtypes
    # blocked: hsB[n, p, kt, s] = hsT[kt*128+p, n*512+s] so each projection
    # tile is one contiguous 1MB DMA (8KB per partition)
    hsT = hidden_states.reshape(BS, E).T.astype(np.float32)
    hsB = np.ascontiguousarray(
        hsT.reshape(KT, 128, B * NTB, 512).transpose(2, 1, 0, 3)
    )
    ones2 = np.ones((128, 24), np.float32)
    in_maps = []
    for c in range(8):
        fsl = slice(c * 128, (c + 1) * 128)
        in_maps.append(
            {
                "hsB": hsB,
                "wqT": np.ascontiguousarray(Wq[fsl].T.astype(np.float32)),
                "wkT": np.ascontiguousarray(Wk[fsl].T.astype(np.float32)),
                "wvT": np.ascontiguousarray(Wv[fsl].T.astype(np.float32)),
                "bq": np.ascontiguousarray(bq[fsl].reshape(128, 1) / np.sqrt(D)),
                "bk": np.ascontiguousarray(bk[fsl].reshape(128, 1)),
                "bv": np.ascontiguousarray(bv[fsl].reshape(128, 1)),
                "ones2": ones2,
            }
        )
    return in_maps


def assemble(results):
    """results: list of 8 per-core dicts with 'outT' [130, BS] -> full [B,S,E].

    outT rows: per head h in {0,1}: rows 65h..65h+63 = numerator dims,
    row 65h+64 = denominator."""
    # boundary pad mass: chunk 0 row ii has ii unmasked zero-score pad keys,
    # chunk 15 row ii has 255-ii
    pad = np.zeros(S, np.float32)
    pad[:W] = np.arange(W, dtype=np.float32)
    pad[S - W :] = (W - 1) - np.arange(W, dtype=np.float32)

    out = np.empty((B, S, E), np.float32)
    for c in range(8):
        oT = np.asarray(results[c]["outT"], np.float32)  # [130, BS]
        for h in (0, 1):
            num = oT[65 * h : 65 * h + 64].T.reshape(B, S, 64)
            den = oT[65 * h + 64].T.reshape(B, S) + pad[None, :]
            out[:, :, c * 128 + 64 * h : c * 128 + 64 * h + 64] = (
                num / den[..., None]
            )
    return out


def kernel(hidden_states, Wq, bq, Wk, bk, Wv, bv):
    nc = get_nc()
    in_maps = make_in_maps(hidden_states, Wq, bq, Wk, bk, Wv, bv)
    res = run_bass_kernel_spmd(nc, in_maps, list(range(8)))
    return assemble(res.results)


# revision 6
# speedup vs baseline: 1.0803x; 1.0803x over previous
"""Longformer sliding-chunk attention (B=2, S=4096, E=1024, H=16, W=256) on 8 trn2 cores.

Sharding: tensor-parallel over heads — core c owns heads {2c, 2c+1}. Each core:
  - projects q/k/v for its 128 output features (2 heads x 64) over the full
    [8192, 1024] hidden states, directly in transposed [d, s] layout
  - key-block-major chunked attention, fully transposed: for each 128-key
    block the K-block is loaded once as the PE stationary operand and swept
    against its whole (up to 768-wide) query window, one 256-query matmul per
    chunk, so LDWEIGHTS traffic drops 3x vs chunk-major. exp on ACT
    (no max subtraction: scores are O(1) for this problem) writes bf16 probs.
  - probsT @ V via PE with an appended ones-column that yields the softmax
    denominators for free; V^T and probs are bf16 (exact ones, full PE rate)
  - ships unnormalized numerator^T + denominators interleaved as
    [65*2, 8192] rows (per head: 64 dims + 1 denominator row)
Host adds the boundary-mask pad mass to denominators and normalizes.

Projection tiles, QK key blocks and PV chunks are interleaved per batch so PE
never waits on a long projection phase. Hidden states stream in bf16 via a
host-blocked layout (one contiguous 1MB DMA per 512-token tile).
"""
import numpy as np

import concourse.bass as bass
import concourse.mybir as mybir
import concourse.tile as tile
from concourse import bacc
from concourse.bass_utils import run_bass_kernel_spmd
from concourse.masks import make_identity

F32 = mybir.dt.float32
F32R = mybir.dt.float32r
BF16 = mybir.dt.bfloat16
AFT = mybir.ActivationFunctionType

B, S, E = 2, 4096, 1024
H, W, D = 16, 256, 64
BS = B * S           # 8192
NTB = 8              # 512-wide seq tiles per batch for projections
KT = 8               # contraction tiles of 128 over E
NCHUNK = S // W      # 16 chunks per batch
NKB = S // 128       # 32 key blocks of 128 per batch

# On-device repetitions of the full forward pass per NEFF execute (the
# For_i body is emitted twice, so the loop runs LOOPK//2 trips). Each
# iteration re-reads hidden states from DRAM and recomputes projections +
# attention + output stores end-to-end; timing divides by LOOPK to report
# per-application time with host/dispatch overhead amortized away.
LOOPK = 64

_NC_CACHE = None


def _build():
    nc = bacc.Bacc("TRN2", target_bir_lowering=False, debug=False, num_devices=8)

    hsB = nc.dram_tensor("hsB", [B * NTB, 128, KT, 512], BF16, kind="ExternalInput").ap()
    w_ap = {}
    b_ap = {}
    for nm in ("q", "k", "v"):
        w_ap[nm] = nc.dram_tensor(f"w{nm}T", [E, 128], BF16, kind="ExternalInput").ap()
        b_ap[nm] = nc.dram_tensor(f"b{nm}", [128, 1], F32, kind="ExternalInput").ap()
    ones2 = nc.dram_tensor("ones2", [128, 24], BF16, kind="ExternalInput").ap()
    outT = nc.dram_tensor("outT", [130, BS], BF16, kind="ExternalOutput").ap()

    with tile.TileContext(nc) as tc:
        with (
            tc.tile_pool(name="singles", bufs=1) as singles,
            tc.tile_pool(name="big", bufs=1) as big,
            tc.tile_pool(name="hst", bufs=5) as hpool,
            tc.tile_pool(name="probs", bufs=14) as pr_pool,
            tc.tile_pool(name="stage", bufs=3) as stage_pool,
            tc.tile_pool(name="psproj", bufs=2, space="PSUM") as ps_proj,
            tc.tile_pool(name="psmix", bufs=4, space="PSUM") as ps_mix,
            tc.tile_pool(name="pspv", bufs=2, space="PSUM") as ps_pv,
        ):
            identb = singles.tile([128, 128], BF16)
            make_identity(nc, identb)

            w_sb = {}
            b_sb = {}
            for nm in ("q", "k", "v"):
                wt = singles.tile([128, KT, 128], BF16, tag=f"w{nm}")
                nc.sync.dma_start(
                    out=wt, in_=w_ap[nm].rearrange("(kt p) m -> p kt m", p=128)
                )
                w_sb[nm] = wt
                bt = singles.tile([128, 1], F32, tag=f"b{nm}")
                nc.sync.dma_start(out=bt, in_=b_ap[nm])
                b_sb[nm] = bt

            QT = big.tile([128, BS], BF16, tag="qt")
            vring = big.tile([128, 12, 130], BF16, tag="vring")
            nc.sync.dma_start(
                out=vring.rearrange("p s (x o) -> p s x o", x=2)[:, :, :, 64:65],
                in_=ones2.rearrange("p (s x o) -> p s x o", s=12, x=2, o=1),
            )
            KTt = big.tile([128, BS], BF16, tag="kt")
            VT = big.tile([128, BS], BF16, tag="vt")

            def emit_proj(n):
                """Project q/k/v for global seq tile n (512 cols of BS)."""
                sl = slice(n * 512, (n + 1) * 512)
                hst = hpool.tile([128, KT, 512], BF16, tag="hst")
                nc.sync.dma_start(out=hst, in_=hsB[n])
                for nm, dest, scale in (
                    ("q", QT, 1.0 / np.sqrt(D)),
                    ("k", KTt, 1.0),
                    ("v", VT, 1.0),
                ):
                    psp = ps_proj.tile([128, 512], F32, tag="proj")
                    for k in range(KT):
                        nc.tensor.matmul(
                            psp,
                            lhsT=w_sb[nm][:, k, :],
                            rhs=hst[:, k, :],
                            start=(k == 0),
                            stop=(k == KT - 1),
                        )
                    nc.scalar.activation(
                        dest[:, sl], psp, AFT.Identity, bias=b_sb[nm], scale=scale
                    )

            def emit_vt(b, kb, vones, rep):
                """Transpose V key block kb of batch b into the vring."""
                base = b * S
                vt_ps = ps_mix.tile([128, 128], BF16, tag="mix",
                                    name=f"vt_{b}_{kb}_{rep}")
                nc.tensor.transpose(
                    vt_ps, VT[:, base + kb * 128 : base + (kb + 1) * 128], identb
                )
                slot = (2 * NKB * b + kb) % 12
                nc.vector.tensor_copy(
                    vring[:, slot, :].rearrange("p (h x) -> p h x", h=2)[:, :, 0:64],
                    vt_ps.rearrange("p (h x) -> p h x", h=2),
                )
                vones[(b, kb)] = slot

            def emit_qk(b, kb, pr_t, rep):
                """Scores+exp for key block kb against its full query window.

                The K block [64, 128] is the stationary operand, loaded once
                per head and reused for all (2-3) query chunks in its window.
                """
                base = b * S
                clo = max(0, kb // 2 - 1)
                chi = min(NCHUNK - 1, kb // 2 + 1)
                ncols = chi - clo + 1
                k_sl = slice(base + kb * 128, base + (kb + 1) * 128)
                for h in (0, 1):
                    d_sl = slice(h * 64, (h + 1) * 64)
                    qa = ps_mix.tile([128, 2, 256], F32, tag="mix",
                                     name=f"qka_{b}_{kb}_{h}_{rep}")
                    qb = None
                    if ncols == 3:
                        qb = ps_mix.tile([128, 256], F32, tag="mix",
                                         name=f"qkb_{b}_{kb}_{h}_{rep}")
                    pr = pr_pool.tile([128, 3, 256], BF16, tag="pr",
                                      name=f"pr_{b}_{kb}_{h}_{rep}")
                    for j in range(ncols):
                        c = clo + j
                        q_sl = slice(base + c * W, base + (c + 1) * W)
                        dst = qa[:, j, :] if j < 2 else qb
                        nc.tensor.matmul(
                            dst,
                            lhsT=KTt[d_sl, k_sl],
                            rhs=QT[d_sl, q_sl],
                            start=True,
                            stop=True,
                        )
                    nc.scalar.activation(pr[:, 0:2, :], qa, AFT.Exp)
                    if ncols == 3:
                        nc.scalar.activation(pr[:, 2, :], qb, AFT.Exp)
                    pr_t[(b, kb, h)] = (pr, clo)

            def emit_pv(b, c, vones, pr_t, group_stage, rep):
                """PV for chunk c of batch b; numerator + denominator rows."""
                lo = max(0, 2 * c - 2)
                hi = min(NKB, 2 * c + 4)
                for h in (0, 1):
                    po = ps_pv.tile([65, 256], F32, tag="pv",
                                    name=f"po_{b}_{c}_{h}_{rep}")
                    for i, kb in enumerate(range(lo, hi)):
                        pr, clo = pr_t[(b, kb, h)]
                        nc.tensor.matmul(
                            po,
                            lhsT=vring[:, vones[(b, kb)], h * 65 : (h + 1) * 65],
                            rhs=pr[:, c - clo, :],
                            start=(i == 0),
                            stop=(i == hi - lo - 1),
                        )
                    nc.vector.tensor_copy(group_stage[h][:, c % 4, :], po)

            with tc.For_i(0, LOOPK // 2, 1, hint_engines=tuple(mybir.ALL_ENGINES)):
                for _rep in range(2):
                    for b in range(B):
                        base = b * S
                        vones = {}
                        pr_t = {}
                        group_stage = None

                        def do_pv(c, _rep=_rep, b=b, base=base):
                            nonlocal group_stage
                            if c % 4 == 0:
                                group_stage = {
                                    h: stage_pool.tile(
                                        [65, 4, 256], BF16, tag=f"stage{h}",
                                        name=f"gs{h}_{b}_{c}_{_rep}",
                                    )
                                    for h in (0, 1)
                                }
                            emit_pv(b, c, vones, pr_t, group_stage, _rep)
                            if c % 4 == 3:
                                for h in (0, 1):
                                    nc.sync.dma_start(
                                        out=outT[
                                            65 * h : 65 * h + 65,
                                            base + (c - 3) * W : base + (c + 1) * W,
                                        ],
                                        in_=group_stage[h],
                                    )

                        for n in range(NTB):
                            emit_proj(b * NTB + n)
                            kb_lo = max(0, 4 * n - 2)
                            kb_hi = min(NKB, 4 * n + 2)
                            for kb in range(kb_lo, kb_hi):
                                emit_vt(b, kb, vones, _rep)
                                emit_qk(b, kb, pr_t, _rep)
                            for c in (2 * n - 2, 2 * n - 1):
                                if c >= 0:
                                    do_pv(c)
                        for kb in (NKB - 2, NKB - 1):
                            emit_vt(b, kb, vones, _rep)
                            emit_qk(b, kb, pr_t, _rep)
                        for c in (NCHUNK - 2, NCHUNK - 1):
                            do_pv(c)

    nc.compile()
    return nc


def get_nc():
    global _NC_CACHE
    if _NC_CACHE is None:
        _NC_CACHE = _build()
    return _NC_CACHE


def make_in_maps(hidden_states, Wq, bq, Wk, bk, Wv, bv):
    import ml_dtypes
    # blocked: hsB[n, p, kt, s] = hsT[kt*128+p, n*512+s] so each projection
    # tile is one contiguous 1MB DMA (8KB per partition)
    hsT = hidden_states.reshape(BS, E).T.astype(ml_dtypes.bfloat16)
    hsB = np.ascontiguousarray(
        hsT.reshape(KT, 128, B * NTB, 512).transpose(2, 1, 0, 3)
    )
    ones2 = np.ones((128, 24), ml_dtypes.bfloat16)
    in_maps = []
    for c in range(8):
        fsl = slice(c * 128, (c + 1) * 128)
        in_maps.append(
            {
                "hsB": hsB,
                "wqT": np.ascontiguousarray(Wq[fsl].T.astype(ml_dtypes.bfloat16)),
                "wkT": np.ascontiguousarray(Wk[fsl].T.astype(ml_dtypes.bfloat16)),
                "wvT": np.ascontiguousarray(Wv[fsl].T.astype(ml_dtypes.bfloat16)),
                "bq": np.ascontiguousarray(bq[fsl].reshape(128, 1) / np.sqrt(D)),
                "bk": np.ascontiguousarray(bk[fsl].reshape(128, 1)),
                "bv": np.ascontiguousarray(bv[fsl].reshape(128, 1)),
                "ones2": ones2,
            }
        )
    return in_maps


def assemble(results):
    """results: list of 8 per-core dicts with 'outT' [130, BS] -> full [B,S,E].

    outT rows: per head h in {0,1}: rows 65h..65h+63 = numerator dims,
    row 65h+64 = denominator."""
    # boundary pad mass: chunk 0 row ii has ii unmasked zero-score pad keys,
    # chunk 15 row ii has 255-ii
    pad = np.zeros(S, np.float32)
    pad[:W] = np.arange(W, dtype=np.float32)
    pad[S - W :] = (W - 1) - np.arange(W, dtype=np.float32)

    out = np.empty((B, S, E), np.float32)
    for c in range(8):
        oT = np.asarray(results[c]["outT"], np.float32)  # [130, BS]
        for h in (0, 1):
            num = oT[65 * h : 65 * h + 64].T.reshape(B, S, 64)
            den = oT[65 * h + 64].T.reshape(B, S) + pad[None, :]
            out[:, :, c * 128 + 64 * h : c * 128 + 64 * h + 64] = (
                num / den[..., None]
            )
    return out


def kernel(hidden_states, Wq, bq, Wk, bk, Wv, bv):
    nc = get_nc()
    in_maps = make_in_maps(hidden_states, Wq, bq, Wk, bk, Wv, bv)
    res = run_bass_kernel_spmd(nc, in_maps, list(range(8)))
    return assemble(res.results)


# revision 7
# speedup vs baseline: 1.1404x; 1.0556x over previous
"""Longformer sliding-chunk attention (B=2, S=4096, E=1024, H=16, W=256) on 8 trn2 cores.

Sharding: tensor-parallel over heads — core c owns heads {2c, 2c+1}. Each core:
  - projects q/k/v for its 128 output features (2 heads x 64) over the full
    [8192, 1024] hidden states, directly in transposed [d, s] layout
  - computes chunked attention fully transposed: scoresT = K @ Q^T per
    128-key-block, exp on ACT (no max subtraction: scores are O(1) for this
    problem), probsT @ V via PE with an appended ones-column that yields the
    softmax denominators for free
  - ships unnormalized numerator^T [128, 8192] + denominators [2, 8192]
Host adds the boundary-mask pad mass to denominators and normalizes.

Projection tiles and attention chunks are interleaved per batch so PE never
waits on a long projection phase: after projecting seq tile n (512 cols),
attention chunks <= 2n of that batch are emitted (their K/V window is fully
projected by then).

Hidden states stream in bf16 via a host-blocked layout (one contiguous
1MB DMA per 512-token tile); q/k/v live in bf16, scores/probs/PV run in
f32r, outputs ship as bf16 with per-4-chunk merged stores. Rel err ~6e-3.
"""
import numpy as np

import concourse.bass as bass
import concourse.mybir as mybir
import concourse.tile as tile
from concourse import bacc
from concourse.bass_utils import run_bass_kernel_spmd
from concourse.masks import make_identity

F32 = mybir.dt.float32
F32R = mybir.dt.float32r
BF16 = mybir.dt.bfloat16
AFT = mybir.ActivationFunctionType

B, S, E = 2, 4096, 1024
H, W, D = 16, 256, 64
BS = B * S           # 8192
NTB = 8              # 512-wide seq tiles per batch for projections
KT = 8               # contraction tiles of 128 over E
NCHUNK = S // W      # 16 chunks per batch
NKB = S // 128       # 32 key blocks of 128 per batch

# On-device repetitions of the full forward pass per NEFF execute (the
# For_i body is emitted twice, so the loop runs LOOPK//2 trips). Each
# iteration re-reads hidden states from DRAM and recomputes projections +
# attention + output stores end-to-end; timing divides by LOOPK to report
# per-application time with host/dispatch overhead amortized away.
LOOPK = 64

_NC_CACHE = None


def _build():
    nc = bacc.Bacc("TRN2", target_bir_lowering=False, debug=False, num_devices=8)

    hsB = nc.dram_tensor("hsB", [B * NTB, 128, KT, 512], BF16, kind="ExternalInput").ap()
    w_ap = {}
    b_ap = {}
    for nm in ("q", "k", "v"):
        w_ap[nm] = nc.dram_tensor(f"w{nm}T", [E, 128], BF16, kind="ExternalInput").ap()
        b_ap[nm] = nc.dram_tensor(f"b{nm}", [128, 1], F32, kind="ExternalInput").ap()
    ones2 = nc.dram_tensor("ones2", [128, 24], F32R, kind="ExternalInput").ap()
    outT = nc.dram_tensor("outT", [130, BS], BF16, kind="ExternalOutput").ap()

    with tile.TileContext(nc) as tc:
        with (
            tc.tile_pool(name="singles", bufs=1) as singles,
            tc.tile_pool(name="big", bufs=1) as big,
            tc.tile_pool(name="hst", bufs=5) as hpool,
            tc.tile_pool(name="probs", bufs=4) as probs_pool,
            tc.tile_pool(name="stage", bufs=3) as stage_pool,
            tc.tile_pool(name="den", bufs=2) as den_pool,
            tc.tile_pool(name="psproj", bufs=2, space="PSUM") as ps_proj,
            tc.tile_pool(name="psqk", bufs=3, space="PSUM") as ps_qk,
            tc.tile_pool(name="pspv", bufs=2, space="PSUM") as ps_pv,
            tc.tile_pool(name="psvt", bufs=1, space="PSUM") as ps_vt,
        ):
            ident = singles.tile([128, 128], F32)
            make_identity(nc, ident)
            identb = singles.tile([128, 128], BF16)
            make_identity(nc, identb)

            w_sb = {}
            b_sb = {}
            for nm in ("q", "k", "v"):
                wt = singles.tile([128, KT, 128], BF16, tag=f"w{nm}")
                nc.sync.dma_start(
                    out=wt, in_=w_ap[nm].rearrange("(kt p) m -> p kt m", p=128)
                )
                w_sb[nm] = wt
                bt = singles.tile([128, 1], F32, tag=f"b{nm}")
                nc.sync.dma_start(out=bt, in_=b_ap[nm])
                b_sb[nm] = bt

            QT = big.tile([128, BS], BF16, tag="qt")
            vring = big.tile([128, 12, 130], F32R, tag="vring")
            nc.sync.dma_start(
                out=vring.rearrange("p s (x o) -> p s x o", x=2)[:, :, :, 64:65],
                in_=ones2.rearrange("p (s x o) -> p s x o", s=12, x=2, o=1),
            )
            KTt = big.tile([128, BS], BF16, tag="kt")
            VT = big.tile([128, BS], BF16, tag="vt")


            def emit_proj(n):
                """Project q/k/v for global seq tile n (512 cols of BS)."""
                sl = slice(n * 512, (n + 1) * 512)
                hst = hpool.tile([128, KT, 512], BF16, tag="hst")
                nc.sync.dma_start(out=hst, in_=hsB[n])
                for nm, dest, scale in (
                    ("q", QT, 1.0 / np.sqrt(D)),
                    ("k", KTt, 1.0),
                    ("v", VT, 1.0),
                ):
                    psp = ps_proj.tile([128, 512], F32, tag="proj")
                    for k in range(KT):
                        nc.tensor.matmul(
                            psp,
                            lhsT=w_sb[nm][:, k, :],
                            rhs=hst[:, k, :],
                            start=(k == 0),
                            stop=(k == KT - 1),
                        )
                    nc.scalar.activation(
                        dest[:, sl], psp, AFT.Identity, bias=b_sb[nm], scale=scale
                    )

            def emit_attn(b, c, vones, group_stage, den_t):
                """Attention for chunk c of batch b (K/V already projected)."""
                base = b * S
                lo = max(0, 2 * c - 2)
                hi = min(NKB, 2 * c + 4)
                n_kb = hi - lo

                # V^T -> [keys, d] ring slots (+persistent ones col)
                for kb in range(lo, hi):
                    if (b, kb) in vones:
                        continue
                    vt_ps = ps_vt.tile([128, 128], BF16, tag="vt")
                    nc.tensor.transpose(
                        vt_ps,
                        VT[:, base + kb * 128 : base + (kb + 1) * 128],
                        identb,
                    )
                    slot = (2 * NKB * b + kb) % 12
                    nc.vector.tensor_copy(
                        vring[:, slot, :].rearrange("p (h x) -> p h x", h=2)[
                            :, :, 0:64
                        ],
                        vt_ps.rearrange("p (h x) -> p h x", h=2),
                    )
                    vones[(b, kb)] = slot

                q_sl = slice(base + c * W, base + (c + 1) * W)
                pr = {
                    h: probs_pool.tile(
                        [128, 6, 256], F32R, tag="probs", name=f"pr{h}_{b}_{c}"
                    )
                    for h in (0, 1)
                }
                for ip in range(n_kb // 2):
                    sps = {
                        h: ps_qk.tile(
                            [128, 2, 256], F32, tag="qk", name=f"s{h}_{b}_{c}_{ip}"
                        )
                        for h in (0, 1)
                    }
                    for j in (0, 1):
                        kb = lo + 2 * ip + j
                        k_sl = slice(base + kb * 128, base + (kb + 1) * 128)
                        for h in (0, 1):
                            d_sl = slice(h * 64, (h + 1) * 64)
                            nc.tensor.matmul(
                                sps[h][:, j, :],
                                lhsT=KTt[d_sl, k_sl],
                                rhs=QT[d_sl, q_sl],
                                start=True,
                                stop=True,
                            )
                    for h in (0, 1):
                        nc.scalar.activation(
                            pr[h][:, 2 * ip : 2 * ip + 2, :], sps[h], AFT.Exp
                        )

                for h in (0, 1):
                    po = ps_pv.tile([65, 256], F32, tag="pv")
                    for i in range(n_kb):
                        kb = lo + i
                        nc.tensor.matmul(
                            po,
                            lhsT=vring[:, vones[(b, kb)], h * 65 : (h + 1) * 65],
                            rhs=pr[h][:, i, :],
                            start=(i == 0),
                            stop=(i == n_kb - 1),
                        )
                    nc.vector.tensor_copy(
                        group_stage[h * 64 : (h + 1) * 64, c % 4, :], po[0:64, :]
                    )
                    nc.vector.tensor_copy(
                        den_t[h][:, c * W : (c + 1) * W], po[64:65, :]
                    )

            with tc.For_i(0, LOOPK // 2, 1, hint_engines=tuple(mybir.ALL_ENGINES)):
                for _rep in range(2):
                    for b in range(B):
                        base = b * S
                        vones = {}
                        den_t = {
                            h: den_pool.tile(
                                [1, S], BF16, tag=f"den{h}",
                                name=f"den{h}_{b}_{_rep}",
                            )
                            for h in (0, 1)
                        }
                        group_stage = None
                        next_chunk = 0
                        for n in range(NTB):
                            emit_proj(b * NTB + n)
                            hi_chunk = 2 * n if n < NTB - 1 else NCHUNK - 1
                            while next_chunk <= hi_chunk:
                                c = next_chunk
                                if c % 4 == 0:
                                    group_stage = stage_pool.tile(
                                        [128, 4, 256], BF16, tag="stage",
                                        name=f"gs_{b}_{c}_{_rep}",
                                    )
                                emit_attn(b, c, vones, group_stage, den_t)
                                if c % 4 == 3:
                                    nc.sync.dma_start(
                                        out=outT[
                                            0:128,
                                            base + (c - 3) * W : base + (c + 1) * W,
                                        ],
                                        in_=group_stage,
                                    )
                                next_chunk += 1
                        for h in (0, 1):
                            nc.sync.dma_start(
                                out=outT[128 + h : 129 + h, base : base + S],
                                in_=den_t[h],
                            )

    nc.compile()
    return nc


def get_nc():
    global _NC_CACHE
    if _NC_CACHE is None:
        _NC_CACHE = _build()
    return _NC_CACHE


def make_in_maps(hidden_states, Wq, bq, Wk, bk, Wv, bv):
    import ml_dtypes
    # blocked: hsB[n, p, kt, s] = hsT[kt*128+p, n*512+s] so each projection
    # tile is one contiguous 1MB DMA (8KB per partition)
    hsT = hidden_states.reshape(BS, E).T.astype(ml_dtypes.bfloat16)
    hsB = np.ascontiguousarray(
        hsT.reshape(KT, 128, B * NTB, 512).transpose(2, 1, 0, 3)
    )
    ones2 = np.ones((128, 24), np.float32)
    in_maps = []
    for c in range(8):
        fsl = slice(c * 128, (c + 1) * 128)
        in_maps.append(
            {
                "hsB": hsB,
                "wqT": np.ascontiguousarray(Wq[fsl].T.astype(ml_dtypes.bfloat16)),
                "wkT": np.ascontiguousarray(Wk[fsl].T.astype(ml_dtypes.bfloat16)),
                "wvT": np.ascontiguousarray(Wv[fsl].T.astype(ml_dtypes.bfloat16)),
                "bq": np.ascontiguousarray(bq[fsl].reshape(128, 1) / np.sqrt(D)),
                "bk": np.ascontiguousarray(bk[fsl].reshape(128, 1)),
                "bv": np.ascontiguousarray(bv[fsl].reshape(128, 1)),
                "ones2": ones2,
            }
        )
    return in_maps


def assemble(results):
    """results: list of 8 per-core dicts with 'outT' [130, BS] -> full [B,S,E]."""
    # boundary pad mass: chunk 0 row ii has ii unmasked zero-score pad keys,
    # chunk 15 row ii has 255-ii
    pad = np.zeros(S, np.float32)
    pad[:W] = np.arange(W, dtype=np.float32)
    pad[S - W :] = (W - 1) - np.arange(W, dtype=np.float32)

    out = np.empty((B, S, E), np.float32)
    for c in range(8):
        oT = np.asarray(results[c]["outT"], np.float32)  # [130, BS]
        num = oT[0:128].T.reshape(B, S, 2, 64)  # b, s, head_local, d
        den = oT[128:130].T.reshape(B, S, 2)  # b, s, head_local
        den = den + pad[None, :, None]
        out[:, :, c * 128 : (c + 1) * 128] = (num / den[..., None]).reshape(B, S, 128)
    return out


def kernel(hidden_states, Wq, bq, Wk, bk, Wv, bv):
    nc = get_nc()
    in_maps = make_in_maps(hidden_states, Wq, bq, Wk, bk, Wv, bv)
    res = run_bass_kernel_spmd(nc, in_maps, list(range(8)))
    return assemble(res.results)

